# revision 23
# baseline (speedup 1.0000x reference)
"""GCN layer (x @ W -> edge gather/scale/scatter-add -> +bias, relu) on 8 NeuronCores.

Measured ~440-570 us/iteration (repeat-slope, noisy env) vs ~830 us for the
fp32 v1 baseline; rel err 3.2e-03 vs the fp32 jax reference (gate 2e-2).

Design:
  - Nodes sharded 8x6250. Per-core bf16 GEMM (x host-converted to bf16,
    fp32 PSUM) -> xw shard stored row-major [6250, 64] bf16 -> AllGather
    builds the full table [50000, 64] bf16 (6.4MB, Shared scratchpad).
  - Parity-packed gather: the table is viewed as [25000, 128] so each
    256-byte dma_gather element (the minimum) holds a NODE PAIR. Edges are
    bucketed by (dst window of 128, src&1); idx = src>>1 fits int16; the
    parity picks which 64-col half feeds the PE. Buckets padded to
    128-multiples, max over cores (~12%), so one SPMD instruction stream.
  - Scatter per 128-edge group: one fused DVE tensor_scalar builds
    onehot*val bf16 ([128 edges, 128 dst], iota==dst * val; scalars must be
    fp32), one bf16 matmul accumulates psum[64, 128] per window (bf16 is
    1 cycle/row on the PE vs 4 for fp32 - the main win over v1). Scalar
    engine applies bias+relu from PSUM into the output stage.
  - The xw table is double-buffered across repeat bodies so the AllGather
    of iteration r+1 overlaps iteration r's gathers (otherwise the WAR on
    the table exposes the collective serially in slope timing).
  - Phase costs (measured by repeat-slope ablation): gather ~200us
    (112k descriptors x 256B, near the ~1.4ns/edge descriptor floor; <512B
    descriptors pay a 2x RMW penalty), all compute only ~62us, head
    (GEMM+AllGather) ~100us and hidden in steady state.
"""

import os
import sys

import numpy as np


def _ensure_concourse():
    try:
        import concourse  # noqa: F401
        return
    except ImportError:
        pass
    for p in ("/opt/trn_rl_repo", "/root/.axon_site/_ro/trn_rl_repo"):
        if os.path.isdir(p):
            sys.path.insert(0, p)
            try:
                import concourse  # noqa: F401
                return
            except ImportError:
                sys.path.pop(0)
    raise ImportError("concourse (bass) not found")


_ensure_concourse()

import ml_dtypes  # noqa: E402

import concourse.bacc as bacc  # noqa: E402
import concourse.mybir as mybir  # noqa: E402
import concourse.tile as tile  # noqa: E402
from concourse import bass_utils  # noqa: E402

F32 = mybir.dt.float32
BF16 = mybir.dt.bfloat16
I16 = mybir.dt.int16
I32 = mybir.dt.int32
BF = ml_dtypes.bfloat16


def _cdiv(a, b):
    return -(-a // b)


def preprocess(edge_src, edge_dst, edge_vals, *, n_nodes, cores, win,
               sort_src=True):
    """Partition/sort/pad edges into per-core parity streams.

    Streams keyed "lo" (src even) / "hi" (src odd); idx is src>>1 into the
    [n_nodes//2, 128] packed bf16 table; parity picks the 64-col half.
    """
    shard = n_nodes // cores
    nwin = _cdiv(shard, win)

    src = np.asarray(edge_src).astype(np.int64)
    dst = np.asarray(edge_dst).astype(np.int64)
    vals = np.asarray(edge_vals).astype(np.float32)
    e = src.shape[0]

    core = dst // shard
    dl = dst - core * shard
    w = dl // win
    h = src & 1
    key = (core * nwin + w) * 2 + h

    order = (np.lexsort((src, key)) if sort_src
             else np.argsort(key, kind="stable"))
    ks = key[order]
    src_s = src[order]
    dloc_s = (dl - w * win)[order].astype(np.float32)
    v_s = vals[order]
    c_s = core[order]
    w_s = w[order]
    h_s = h[order]

    nbuck = cores * nwin * 2
    sizes = np.bincount(key, minlength=nbuck)
    starts = np.concatenate(([0], np.cumsum(sizes)))[:-1]
    rank = np.arange(e, dtype=np.int64) - starts[ks]

    # groups per (window, parity): max over cores
    cnt = sizes.reshape(cores, nwin, 2)
    G = _cdiv(cnt, 128).max(axis=0)  # [nwin, 2]
    glo, ghi = G[:, 0], G[:, 1]
    cum_lo = np.concatenate(([0], np.cumsum(glo)))  # group offsets per window
    cum_hi = np.concatenate(([0], np.cumsum(ghi)))
    gtot_lo, gtot_hi = int(cum_lo[-1]), int(cum_hi[-1])
    nlo, nhi = gtot_lo * 128, gtot_hi * 128

    idx_lo = np.zeros((cores, max(nlo, 1)), np.int16)
    dst_lo = np.zeros((cores, max(nlo, 1)), np.float32)
    val_lo = np.zeros((cores, max(nlo, 1)), np.float32)
    idx_hi = np.zeros((cores, max(nhi, 1)), np.int16)
    dst_hi = np.zeros((cores, max(nhi, 1)), np.float32)
    val_hi = np.zeros((cores, max(nhi, 1)), np.float32)

    m = h_s == 0
    pos = cum_lo[w_s[m]] * 128 + rank[m]
    idx_lo[c_s[m], pos] = (src_s[m] >> 1).astype(np.int16)
    dst_lo[c_s[m], pos] = dloc_s[m]
    val_lo[c_s[m], pos] = v_s[m]

    m = h_s == 1
    pos = cum_hi[w_s[m]] * 128 + rank[m]
    idx_hi[c_s[m], pos] = (src_s[m] >> 1).astype(np.int16)
    dst_hi[c_s[m], pos] = dloc_s[m]
    val_hi[c_s[m], pos] = v_s[m]

    def idx_layout(a, n):
        # logical position i -> [i % 16, i // 16], replicated to 128 partitions
        if n == 0:
            return None
        blk = a[:n].reshape(-1, 16).T  # [16, n/16]
        return np.ascontiguousarray(np.tile(blk, (8, 1)))  # [128, n/16]

    def grp_layout(a, n):
        # position g*128+p -> [p, g] (fp32: tensor_scalar scalars must be f32)
        if n == 0:
            return None
        return np.ascontiguousarray(a[:n].reshape(-1, 128).T)

    return dict(
        shard=shard,
        nwin=nwin,
        glo=glo,
        ghi=ghi,
        cum_lo=cum_lo,
        cum_hi=cum_hi,
        gtot_lo=gtot_lo,
        gtot_hi=gtot_hi,
        nlo=nlo,
        nhi=nhi,
        idx_lo=[idx_layout(idx_lo[c], nlo) for c in range(cores)],
        dst_lo=[grp_layout(dst_lo[c], nlo) for c in range(cores)],
        val_lo=[grp_layout(val_lo[c], nlo) for c in range(cores)],
        idx_hi=[idx_layout(idx_hi[c], nhi) for c in range(cores)],
        dst_hi=[grp_layout(dst_hi[c], nhi) for c in range(cores)],
        val_hi=[grp_layout(val_hi[c], nhi) for c in range(cores)],
    )


def build_program(meta, *, n_nodes, din, dout, cores, win=128, maxb=16,
                  msgs_bufs=4, sc_ps_bufs=6, oh_bufs=32, poolk=0, flip=False,
                  copy_msgs=12, single_packet=False, ablate=None, repeat=1,
                  repeat_part="all"):
    """Build the SPMD Bass program. repeat_part in {"all","scatter","head"}
    controls which phase the repeat>1 bodies re-emit (for slope timing)."""
    shard = meta["shard"]
    nwin = meta["nwin"]
    glo, ghi = meta["glo"], meta["ghi"]
    cum_lo, cum_hi = meta["cum_lo"], meta["cum_hi"]
    gtot_lo, gtot_hi = meta["gtot_lo"], meta["gtot_hi"]
    nlo, nhi = meta["nlo"], meta["nhi"]
    kch = _cdiv(din, 128)
    nr = _cdiv(shard, 128)

    nc = bacc.Bacc("TRN2", target_bir_lowering=False, debug=False,
                   num_devices=cores, num_swdge_queues=4)

    t_xT = nc.dram_tensor("xT", [din, shard], BF16, kind="ExternalInput")
    t_w = nc.dram_tensor("w", [din, dout], BF16, kind="ExternalInput")
    t_bias = nc.dram_tensor("bias", [dout, 1], F32, kind="ExternalInput")
    t_bias_row = (nc.dram_tensor("bias_row", [1, dout], BF16,
                                 kind="ExternalInput") if flip else None)
    t_idx = {}
    t_dst = {}
    t_val = {}
    if nlo:
        t_idx["lo"] = nc.dram_tensor("idx_lo", [128, nlo // 16], I16, kind="ExternalInput")
        t_dst["lo"] = nc.dram_tensor("dst_lo", [128, gtot_lo], F32, kind="ExternalInput")
        t_val["lo"] = nc.dram_tensor("val_lo", [128, gtot_lo], F32, kind="ExternalInput")
    if nhi:
        t_idx["hi"] = nc.dram_tensor("idx_hi", [128, nhi // 16], I16, kind="ExternalInput")
        t_dst["hi"] = nc.dram_tensor("dst_hi", [128, gtot_hi], F32, kind="ExternalInput")
        t_val["hi"] = nc.dram_tensor("val_hi", [128, gtot_hi], F32, kind="ExternalInput")
    t_out = nc.dram_tensor("outT", [shard, dout] if flip else [dout, shard],
                           F32, kind="ExternalOutput")

    nbuf = 2 if repeat > 1 else 1
    t_xw_shards = [nc.dram_tensor(f"xw_shard{i}", [shard, dout], BF16)
                   for i in range(nbuf)]
    t_xw_fulls = [nc.dram_tensor(f"xw_full{i}", [n_nodes, dout], BF16,
                                 addr_space="Shared" if cores > 4 else "Local")
                  for i in range(nbuf)]

    with tile.TileContext(nc) as tc:
        with (
            tc.tile_pool(name="const", bufs=1) as constp,
            tc.tile_pool(name="xt", bufs=1) as xtp,
            tc.tile_pool(name="stage", bufs=1) as stagep,
            tc.tile_pool(name="meta", bufs=1) as metap,
            tc.tile_pool(name="msgs_lo", bufs=msgs_bufs) as mlp,
            tc.tile_pool(name="msgs_hi", bufs=msgs_bufs) as mhp,
            tc.tile_pool(name="cmsg", bufs=max(copy_msgs, 1)) as cmp_,
            tc.tile_pool(name="oh", bufs=oh_bufs) as ohp,
            tc.tile_pool(name="gemm_ps", bufs=2, space="PSUM") as gpsp,
            tc.tile_pool(name="sc_ps", bufs=sc_ps_bufs, space="PSUM") as spsp,
        ):
            # ---- constants ----
            iota_i = constp.tile([128, win], I32)
            nc.gpsimd.iota(iota_i[:], pattern=[[0, 1], [1, win]], base=0,
                           channel_multiplier=0)
            iota_f32 = constp.tile([128, win], F32)
            nc.vector.tensor_copy(iota_f32[:], iota_i[:])
            iota_f = constp.tile([128, win], BF16)
            nc.vector.tensor_copy(iota_f[:], iota_f32[:])
            bias_sb = constp.tile([dout, 1], F32)
            nc.sync.dma_start(bias_sb[:], t_bias[:])
            if flip:
                bias_row = constp.tile([1, dout], BF16)
                nc.sync.dma_start(bias_row[:], t_bias_row[:])
                ones_sb = constp.tile([1, win], BF16)
                nc.vector.memset(ones_sb[:], 1.0)
            w_sb = constp.tile([128, kch * dout], BF16)
            nc.sync.dma_start(
                w_sb[:].rearrange("p (k e) -> p k e", e=dout),
                t_w[:].rearrange("(k p) e -> p k e", p=128),
            )

            # ---- metadata loads ----
            sb_idx = {}
            sb_dst = {}
            sb_val = {}
            for s in t_idx:
                sb_idx[s] = metap.tile(list(t_idx[s].shape), I16, tag=f"idx_{s}", name=f"sb_idx_{s}")
                nc.sync.dma_start(sb_idx[s][:], t_idx[s][:])
                sb_dst[s] = metap.tile(list(t_dst[s].shape), F32, tag=f"dst_{s}", name=f"sb_dst_{s}")
                nc.sync.dma_start(sb_dst[s][:], t_dst[s][:])
                sb_val[s] = metap.tile(list(t_val[s].shape), F32, tag=f"val_{s}", name=f"sb_val_{s}")
                nc.sync.dma_start(sb_val[s][:], t_val[s][:])

            def emit_head(rep, do_collective=True):
                t_xw_shard = t_xw_shards[rep % nbuf]
                t_xw_full = t_xw_fulls[rep % nbuf]
                # ---- local GEMM: xw_shard = x_shard @ W (bf16, fp32 acc) ----
                xt_sb = []
                for k in range(kch):
                    kp = min(128, din - k * 128)
                    xt = xtp.tile([kp, shard], BF16, tag=f"xt{k}")
                    nc.sync.dma_start(xt[:], t_xT[k * 128:k * 128 + kp, :])
                    xt_sb.append(xt)
                xw_stage = stagep.tile([128, nr * dout], BF16, tag="xw_stage")
                for r in range(nr):
                    rw = min(128, shard - r * 128)
                    ps = gpsp.tile([rw, dout], F32, tag="gemm_ps")
                    for k in range(kch):
                        nc.tensor.matmul(
                            ps[:],
                            xt_sb[k][:, r * 128:r * 128 + rw],
                            w_sb[:xt_sb[k].shape[0], k * dout:(k + 1) * dout],
                            start=(k == 0),
                            stop=(k == kch - 1),
                        )
                    nc.scalar.activation(
                        xw_stage[:rw, r * dout:(r + 1) * dout], ps[:],
                        mybir.ActivationFunctionType.Copy)
                # store xw_shard (row-major bf16) then AllGather
                nfull = shard // 128
                nc.sync.dma_start(
                    t_xw_shard[: nfull * 128, :].rearrange("(r p) e -> p r e", p=128),
                    xw_stage[:, : nfull * dout].rearrange("p (r e) -> p r e", e=dout),
                )
                if shard > nfull * 128:
                    rw = shard - nfull * 128
                    nc.sync.dma_start(
                        t_xw_shard[nfull * 128:, :],
                        xw_stage[:rw, nfull * dout:(nfull + 1) * dout],
                    )
                if do_collective:
                    nc.gpsimd.collective_compute(
                        "AllGather",
                        mybir.AluOpType.bypass,
                        replica_groups=[list(range(cores))],
                        ins=[t_xw_shard[:]],
                        outs=[t_xw_full[:]],
                    )

            def emit_scatter(rep, buf=None):
                # packed-table view [n_nodes//2, 2*dout] for the gather
                t_xw_packed = t_xw_fulls[(rep if buf is None else buf)
                                         % nbuf][:].rearrange(
                    "(k two) e -> k (two e)", two=2)
                const_msgs = None
                if ablate in ("no_gather", "const_mm"):
                    const_msgs = stagep.tile([128, maxb * 2 * dout], BF16,
                                             tag="const_msgs")
                    nc.vector.memset(const_msgs[:], 1.0)
                streams = [s for s in ("lo", "hi") if (nlo if s == "lo" else nhi)]
                off = {"lo": 0, "hi": dout}
                gtot = {"lo": gtot_lo, "hi": gtot_hi}
                nbatch = {s: _cdiv(gtot[s], maxb) for s in streams}
                pool = {"lo": mlp, "hi": mhp}
                msgs_buf = {s: [None] * nbatch[s] for s in streams}
                qctr = [0]

                def emit_gather(s, b):
                    g0 = b * maxb
                    gn = min(maxb, gtot[s] - g0)
                    n_idx = gn * 128
                    buf = pool[s].tile([128, gn * 2 * dout], BF16, tag=f"msgs_{s}",
                                       name=f"msgs_{s}_{b}_r{rep}")
                    if ablate == "no_gather":
                        msgs_buf[s][b] = const_msgs
                        return
                    nc.gpsimd.dma_gather(
                        buf[:].rearrange("p (c e) -> p c e", e=2 * dout),
                        t_xw_packed,
                        sb_idx[s][:, g0 * 8:(g0 + gn) * 8],
                        n_idx,
                        n_idx,
                        2 * dout,
                        single_packet=single_packet,
                        queue_num=qctr[0] % 4,
                    )
                    qctr[0] += 1
                    if copy_msgs:
                        # settle the batch on the (idle) scalar engine so the
                        # PE never reads a tile with in-flight DMA writes
                        cbuf = cmp_.tile([128, gn * 2 * dout], BF16,
                                         tag="cmsg", name=f"cmsg_{s}_{b}_r{rep}")
                        nc.scalar.activation(
                            cbuf[:], buf[:],
                            mybir.ActivationFunctionType.Copy)
                        msgs_buf[s][b] = cbuf
                        return
                    msgs_buf[s][b] = buf

                out_stage = stagep.tile(
                    [128, nwin * dout] if flip else [dout, shard],
                    F32, tag="out_stage")
                cum = {"lo": cum_lo, "hi": cum_hi}
                for wi in range(nwin):
                    ww = min(win, shard - wi * win)
                    spans = [(s, int(cum[s][wi]), int(cum[s][wi + 1])) for s in streams]
                    ngrp = sum(g1 - g0 for _, g0, g1 in spans)
                    if not flip and ngrp == 0:
                        zps = spsp.tile([dout, win], F32, tag="sc_ps")
                        nc.vector.memset(zps[:], 0.0)
                        nc.scalar.activation(
                            out_stage[:, wi * win:wi * win + ww], zps[:, :ww],
                            mybir.ActivationFunctionType.Relu, bias=bias_sb[:],
                        )
                        continue
                    if ablate in ("gather_only", "no_mm"):
                        gi = 0
                        for s, g0, g1 in spans:
                            for g in range(g0, g1):
                                b, j = g // maxb, g % maxb
                                if msgs_buf[s][b] is None:
                                    emit_gather(s, b)
                                if ablate == "no_mm":
                                    oh = ohp.tile([128, win], BF16, tag="oh")
                                    nc.vector.tensor_scalar(
                                        oh[:], iota_f[:],
                                        sb_dst[s][:, g:g + 1],
                                        sb_val[s][:, g:g + 1],
                                        mybir.AluOpType.is_equal,
                                        mybir.AluOpType.mult,
                                    )
                                gi += 1
                        continue
                    ps = spsp.tile([win, dout] if flip else [dout, win],
                                   F32, tag="sc_ps")
                    gi = 0
                    if flip:
                        # psum <- ones.T @ bias_row (exact bias broadcast)
                        nc.tensor.matmul(ps[:], ones_sb[:], bias_row[:],
                                         start=True, stop=(ngrp == 0))
                    for s, g0, g1 in spans:
                        for g in range(g0, g1):
                            b, j = g // maxb, g % maxb
                            if msgs_buf[s][b] is None:
                                emit_gather(s, b)
                            if ablate in ("gather_only", "no_mm"):
                                if ablate == "no_mm":
                                    oh = ohp.tile([128, win], BF16, tag="oh")
                                    nc.vector.tensor_scalar(
                                        oh[:], iota_f[:],
                                        sb_dst[s][:, g:g + 1],
                                        sb_val[s][:, g:g + 1],
                                        mybir.AluOpType.is_equal,
                                        mybir.AluOpType.mult,
                                    )
                                gi += 1
                                continue
                            if ablate == "no_oh":
                                oh = None
                                oh_ap = iota_f[:]
                            else:
                                oh = ohp.tile([128, win], BF16, tag="oh")
                                eng = (nc.gpsimd
                                       if poolk and gi % poolk == poolk - 1
                                       else nc.vector)
                                eng.tensor_scalar(
                                    oh[:],
                                    iota_f[:],
                                    sb_dst[s][:, g:g + 1],
                                    sb_val[s][:, g:g + 1],
                                    mybir.AluOpType.is_equal,
                                    mybir.AluOpType.mult,
                                )
                                oh_ap = oh[:]
                            mbuf = (const_msgs if ablate == "const_mm"
                                    else msgs_buf[s][b])
                            msl = mbuf[:, j * 2 * dout + off[s]:
                                       j * 2 * dout + off[s] + dout]
                            if flip:
                                nc.tensor.matmul(
                                    ps[:], oh_ap, msl,
                                    start=False, stop=(gi == ngrp - 1),
                                )
                            else:
                                nc.tensor.matmul(
                                    ps[:], msl, oh_ap,
                                    start=(gi == 0), stop=(gi == ngrp - 1),
                                )
                            gi += 1
                    if flip:
                        nc.scalar.activation(
                            out_stage[:ww, wi * dout:(wi + 1) * dout],
                            ps[:ww, :],
                            mybir.ActivationFunctionType.Relu,
                        )
                    else:
                        nc.scalar.activation(
                            out_stage[:, wi * win:wi * win + ww], ps[:, :ww],
                            mybir.ActivationFunctionType.Relu, bias=bias_sb[:],
                        )
                if ablate in ("gather_only", "no_mm"):
                    return
                if flip:
                    nfull = shard // 128
                    nc.sync.dma_start(
                        t_out[: nfull * 128, :].rearrange(
                            "(r p) e -> p r e", p=128),
                        out_stage[:, : nfull * dout].rearrange(
                            "p (r e) -> p r e", e=dout),
                    )
                    if shard > nfull * 128:
                        rw = shard - nfull * 128
                        nc.sync.dma_start(
                            t_out[nfull * 128:, :],
                            out_stage[:rw, nfull * dout:(nfull + 1) * dout],
                        )
                else:
                    nc.sync.dma_start(t_out[:], out_stage[:])

            if repeat_part == "all":
                for _rep in range(repeat):
                    emit_head(_rep)
                    emit_scatter(_rep)
            elif repeat_part == "scatter":
                emit_head(0)
                for _rep in range(repeat):
                    emit_scatter(_rep, buf=0)
            elif repeat_part == "head":
                for _rep in range(repeat):
                    emit_head(_rep)
                emit_scatter(0)
            elif repeat_part == "gemm":
                for _rep in range(repeat):
                    emit_head(_rep, do_collective=False)
            else:
                raise ValueError(repeat_part)

    nc.compile()
    return nc


def run(inputs, *, n_nodes, n_edges, din, dout, cores, win=128, maxb=16,
        msgs_bufs=4, sc_ps_bufs=6, oh_bufs=32, poolk=0, flip=False,
        copy_msgs=12, trace=False, repeat=1, repeat_part="all"):
    x = np.asarray(inputs["x"], dtype=np.float32)
    weight = np.asarray(inputs["weight"], dtype=np.float32).astype(BF)
    bias = np.ascontiguousarray(
        np.asarray(inputs["bias"], dtype=np.float32).reshape(dout, 1))
    meta = preprocess(
        inputs["edge_src"], inputs["edge_dst"], inputs["edge_vals"],
        n_nodes=n_nodes, cores=cores, win=win)
    shard = meta["shard"]

    nc = build_program(meta, n_nodes=n_nodes, din=din, dout=dout, cores=cores,
                       win=win, maxb=maxb, msgs_bufs=msgs_bufs,
                       sc_ps_bufs=sc_ps_bufs, oh_bufs=oh_bufs, poolk=poolk,
                       flip=flip, copy_msgs=copy_msgs,
                       repeat=repeat, repeat_part=repeat_part)

    xT = np.ascontiguousarray(x.T.astype(BF))
    in_maps = []
    for c in range(cores):
        m = {
            "xT": np.ascontiguousarray(xT[:, c * shard:(c + 1) * shard]),
            "w": weight,
            "bias": bias,
        }
        if flip:
            m["bias_row"] = np.ascontiguousarray(bias.reshape(1, dout).astype(BF))
        if meta["nlo"]:
            m["idx_lo"] = meta["idx_lo"][c]
            m["dst_lo"] = meta["dst_lo"][c]
            m["val_lo"] = meta["val_lo"][c]
        if meta["nhi"]:
            m["idx_hi"] = meta["idx_hi"][c]
            m["dst_hi"] = meta["dst_hi"][c]
            m["val_hi"] = meta["val_hi"][c]
        in_maps.append(m)

    res = bass_utils.run_bass_kernel_spmd(
        nc, in_maps, core_ids=list(range(cores)), trace=trace)
    out = np.concatenate(
        [res.results[c]["outT"] if flip else res.results[c]["outT"].T
         for c in range(cores)], axis=0)
    run.last_nc = nc
    run.last_in_maps = in_maps
    return out, res


def kernel(**inputs):
    out, _ = run(
        inputs,
        n_nodes=50000, n_edges=800000, din=256, dout=64, cores=8,
    )
    return np.ascontiguousarray(out, dtype=np.float32)
